# revision 66
# baseline (speedup 1.0000x reference)
"""Trainium2 Bass kernel for policy-masked attention (sparse_attention).

Shapes (hardcoded): x [4,1024,768], decision [4,768,2], qkv_w [2304,768],
qkv_b [2304], proj_w [768,768], proj_b [768], search_feat_len=768.

Sharding: 8 cores = 4 batches x 2 head-groups (6 heads each). Each core
computes its batch's q/k/v for its heads, the policy-masked softmax
(one-hot policy folded into the score matmul as 2 extra contraction rows
of -BIG * indicator outer products), attn @ v with a fused ones-column
producing the softmax denominator, and a partial output projection.
Host sums the two head-group partials per batch and adds proj_b.

Tokens are permuted host-side into [g0 | g1-tail + g2 | g1-pure] order
(g0 = 256 template tokens, g1/g2 = the two decision groups). Key tiles
0,1 are then pure g0 and tiles 6,7 pure g1, so the mask zeroes entire
scoreblocks which are skipped: key tiles 0,1 skip query columns
768:1024 (g1-pure) and key tiles 6,7 skip query columns 0:256 (g0).
The -BIG mask rows still handle the ragged g1-tail inside tiles 2-5.

The entire PE path runs bf16 (x, weights, q/k, exp output, V, normalized
attention, projection weights): beyond halving DMA and SBUF, bf16
stationaries load via explicit Ldweights which the PE overlaps with the
previous matmul's streaming, while float32r matmuls must self-load their
4-byte stationaries serially — measured ~9% faster end-to-end on
hardware. Accumulation stays fp32 in PSUM; rel err ~4.4e-3 vs the fp32
reference. The softmax denominator reciprocal is broadcast across
partitions with gpsimd partition_broadcast (no DRAM bounce).
"""
import numpy as np
import ml_dtypes

import concourse.bass as bass
import concourse.tile as tile
from concourse import bacc, mybir
from concourse.bass_utils import run_bass_kernel_spmd

F32 = mybir.dt.float32
F32R = mybir.dt.float32r
BF16 = mybir.dt.bfloat16
AF = mybir.ActivationFunctionType
ALU = mybir.AluOpType

B, N, C = 4, 1024, 768
H = 12
HD = 64
HPC = 6              # heads per core
KT = C // 128        # 6 contraction tiles
MT = N // 128        # 8 key tiles
CO = C // 128        # 6 output-column tiles
G0 = 256             # template tokens (N - S); tiles 0,1
SCALE = HD ** -0.5
BIG = 32768.0
EPS = 1e-6
N_CORES = 8

# per key-tile: computed query-column range (cols outside are masked)
KEEP = [(0, 768)] * 2 + [(0, 1024)] * 4 + [(256, 1024)] * 2


def round_fp32r(a: np.ndarray) -> np.ndarray:
    """Round-to-nearest-even to 11-bit mantissa (the PE fp32r format)."""
    bits = np.ascontiguousarray(a, dtype=np.float32).view(np.uint32)
    r = bits + np.uint32(0x7FF) + ((bits >> np.uint32(12)) & np.uint32(1))
    r &= np.uint32(0xFFFFF000)
    return r.view(np.float32)


def _score_chunks(m):
    """Bank-aligned (lo, hi) column chunks of the computed score range."""
    lo, hi = KEEP[m]
    if lo == 0 and hi == 768:
        return [(0, 512), (512, 768)]
    if lo == 256:
        return [(256, 512), (512, 1024)]
    return [(0, 512), (512, 1024)]


def _body(nc, tc, t, with_vbias=True, dummies=False, zrep=False):
    """Emit one full forward pass. t = dict of dram tensor handles.

    zrep: replicate Z across psum partitions 64:128 via 64 ones-columns
    in the AV stationary (bf16 Ldweights make the wider stationary cheap),
    so 1/Z is a single [64,512] DVE reciprocal per half — no gpsimd
    broadcast and no Ust staging in the drain.
    """
    import contextlib
    with contextlib.ExitStack() as ctx:
        consts = ctx.enter_context(tc.tile_pool(name="consts", bufs=1))
        headp = ctx.enter_context(tc.tile_pool(name="headp", bufs=1))

        xT_sb = consts.tile([128, KT, N], BF16)
        wqkT_sb = consts.tile([128, KT, KT, 128], BF16)
        wvT_sb = consts.tile([128, KT, HPC * HD], BF16)
        vbias_sb = consts.tile([1, HPC * HD], BF16)
        ones1_sb = consts.tile([1, 128], BF16)
        qkb_sb = consts.tile([128, KT], F32)
        projT_sb = consts.tile([128, 3, C], BF16)
        pbias_sb = consts.tile([128, CO], F32)

        # Bulk bf16 input streams on the sync queue, in first-use order;
        # small tiles ride the idle gpsimd queue.
        JSEQ = [0, 3, 1, 4, 2, 5]
        nc.sync.dma_start(out=wqkT_sb[:, JSEQ[0]], in_=t["wqkT"].ap()[:, JSEQ[0]])
        for kt in range(KT):
            nc.sync.dma_start(out=xT_sb[:, kt, :], in_=t["xT"].ap()[:, kt, :])
        nc.sync.dma_start(out=wqkT_sb[:, JSEQ[1]], in_=t["wqkT"].ap()[:, JSEQ[1]])
        nc.sync.dma_start(out=wvT_sb, in_=t["wvT"].ap())
        for j in JSEQ[2:]:
            nc.sync.dma_start(out=wqkT_sb[:, j], in_=t["wqkT"].ap()[:, j])
        nc.sync.dma_start(out=projT_sb, in_=t["projT"].ap())
        nc.gpsimd.dma_start(out=qkb_sb, in_=t["qkb"].ap())
        nc.gpsimd.dma_start(out=pbias_sb, in_=t["pbias"].ap())
        if with_vbias:
            nc.gpsimd.dma_start(out=vbias_sb, in_=t["vbias"].ap())
            nc.gpsimd.dma_start(out=ones1_sb, in_=t["ones1"].ap())

        # Per-head q/k tiles [66, N]: rows 0-63 head data, rows 64-65 the
        # rank-2 log-mask factors (k side: -BIG*p0,-BIG*p1; q side: p1,p0).
        # bf16 throughout the PE path: unlike f32r (whose matmuls must
        # self-load their stationary, serializing load->stream), bf16
        # stationaries go through explicit Ldweights which the PE can
        # overlap with the previous matmul's streaming.
        qh = [headp.tile([66, N], BF16, name=f"qh{h}", tag=f"qh{h}")
              for h in range(HPC)]
        kh = [headp.tile([66, N], BF16, name=f"kh{h}", tag=f"kh{h}")
              for h in range(HPC)]
        for h in range(HPC):
            nc.gpsimd.dma_start(out=qh[h][64:66, :], in_=t["mq"].ap())
            nc.gpsimd.dma_start(out=kh[h][64:66, :], in_=t["mk"].ap())

        # V in token-major layout with fused ones column(s): [128, MT, 6*VW]
        VW = 128 if zrep else 65
        V_sb = consts.tile([128, MT, HPC * VW], BF16)
        vv = V_sb.rearrange("p m (h e) -> p m h e", h=HPC)
        nc.vector.memset(vv[:, :, :, 64:VW].bitcast(mybir.dt.uint16),
                         0x3F80)  # 1.0 in the bf16 ones column(s)

        warm = consts.tile([1, 1], F32)
        nc.scalar.activation(warm, qkb_sb[0:1, 0:1], AF.Exp)

        if dummies:
            wdum = consts.tile([128, 512], BF16)
            nc.vector.memset(wdum.bitcast(mybir.dt.uint32), 0)

        def qk_block(j, q_on_act, wmm=None):
            """q,k generation for one 128-row output block (2 heads' halves).

            Block j < 3 is q (pre-scaled), j >= 3 is k. Copies go on the
            scalar engine pre-phase2 (q_on_act) or the vector engine when
            emitted inside the exp-paced attention phase.
            """
            ps = stp.tile([128, N], F32, tag="st", name=f"qkps{j}")
            for kt in range(KT):
                if wmm is not None and kt > 0:
                    wmm(1)
                for n in range(2):
                    nc.tensor.matmul(ps[:, n * 512:(n + 1) * 512],
                                     lhsT=wqkT_sb[:, j, kt, :],
                                     rhs=xT_sb[:, kt, n * 512:(n + 1) * 512],
                                     start=(kt == 0), stop=(kt == KT - 1))
            tiles = qh if j < 3 else kh
            jj = j % 3
            for half in range(2):
                h = 2 * jj + half
                src = ps[half * 64:(half + 1) * 64, :]
                bias = qkb_sb[half * 64:(half + 1) * 64, j:j + 1]
                if q_on_act:
                    nc.scalar.activation(tiles[h][0:64, :], src, AF.Identity,
                                         bias=bias, scale=1.0)
                else:
                    nc.vector.tensor_scalar(out=tiles[h][0:64, :], in0=src,
                                            scalar1=bias, scalar2=None,
                                            op0=ALU.add)

        zpool = ctx.enter_context(tc.tile_pool(name="zpool", bufs=1))
        UstA = zpool.tile([64, 3, N], F32)   # even heads of each pair
        UstB = zpool.tile([64, 3, N], F32)   # odd heads
        # one tile per head-pair so projection reads depend only on the
        # pairs actually consumed (pair 0/1 columns start early)
        Ab = [zpool.tile([128, N], BF16, name=f"Ab{i}", tag=f"Ab{i}")
              for i in range(3)]

        stp = ctx.enter_context(tc.tile_pool(name="stp", bufs=2, space="PSUM"))

        # ---- q/k for heads 0,1; optional PE warm-up matmuls ride the
        # initial DMA trickle so real matmuls issue without data waits ----
        if dummies:
            with tc.tile_pool(name="wp", bufs=1, space="PSUM") as wp:
                wps = wp.tile([128, 512], F32, tag="w")

                def warm_mm(k):
                    for _ in range(k):
                        nc.tensor.matmul(wps, lhsT=wdum[:, 0:128], rhs=wdum,
                                         start=True, stop=True,
                                         skip_group_check=True)

                warm_mm(9)
                qk_block(JSEQ[0], True, wmm=warm_mm)
                qk_block(JSEQ[1], False)
        else:
            qk_block(JSEQ[0], True)
            qk_block(JSEQ[1], False)

        # ---- V in token-major layout (+ bias via ones row) ----
        with tc.tile_pool(name="vp", bufs=2, space="PSUM") as vp:
            for m in range(MT):
                psv = vp.tile([128, HPC * HD], F32, tag="v")
                for kt in range(KT):
                    nc.tensor.matmul(psv,
                                     lhsT=xT_sb[:, kt, m * 128:(m + 1) * 128],
                                     rhs=wvT_sb[:, kt, :],
                                     start=(kt == 0),
                                     stop=(not with_vbias and kt == KT - 1))
                if with_vbias:
                    nc.tensor.matmul(psv, lhsT=ones1_sb, rhs=vbias_sb,
                                     start=False, stop=True)
                nc.vector.tensor_copy(vv[:, m, :, 0:64],
                                      psv.rearrange("p (h d) -> p h d", h=HPC))

        # ---- per-head masked scores, exp, AV(+Z); remaining q/k blocks
        # interleave into the exp-paced pipeline ----
        with tc.tile_pool(name="up", bufs=2, space="PSUM") as up, \
             tc.tile_pool(name="ep", bufs=4) as ep, \
             tc.tile_pool(name="zqp", bufs=2) as zqp, \
             tc.tile_pool(name="zbp", bufs=2) as zbp:
            zq = [None, None]
            for h in range(HPC):
                tt, half = h // 2, h % 2
                uLO = up.tile([VW, 512], F32, tag="uLO", name=f"uLO{h}")
                uHI = up.tile([VW, 512], F32, tag="uHI", name=f"uHI{h}")
                # Full-width key tiles go first so each U bank's initial
                # matmul covers the whole bank with start=True; the skipped
                # quadrants of tiles 0,1,6,7 then join as plain accumulates.
                MSEQ = [2, 3, 4, 5, 0, 1, 6, 7]

                def av(mi, m, e):
                    # A@V accumulation; masked-out 256-col blocks skipped.
                    # uLO = query cols 0:512, uHI = 512:1024.
                    lo, hi = KEEP[m]
                    vh = V_sb[:, m, h * VW:(h + 1) * VW]
                    nc.tensor.matmul(uLO[:, lo:512],
                                     lhsT=vh, rhs=e[:, lo:512],
                                     start=(mi == 0), stop=(mi == MT - 1),
                                     skip_group_check=True)
                    nc.tensor.matmul(uHI[:, 0:hi - 512],
                                     lhsT=vh, rhs=e[:, 512:hi],
                                     start=(mi == 0), stop=(mi == MT - 1),
                                     skip_group_check=True)

                # Software-pipelined: AV for tile m issues behind scores for
                # tile m+1 so the in-order PE stream never waits on exp.
                pending = None
                for mi, m in enumerate(MSEQ):
                    st = stp.tile([128, N], F32, tag="st")
                    lo, hi = KEEP[m]
                    for clo, chi in _score_chunks(m):
                        nc.tensor.matmul(st[:, clo:chi],
                                         lhsT=kh[h][:, m * 128:(m + 1) * 128],
                                         rhs=qh[h][:, clo:chi],
                                         start=True, stop=True)
                    e = ep.tile([128, N], BF16, tag="e")
                    nc.scalar.activation(e[:, lo:hi], st[:, lo:hi], AF.Exp)
                    if pending is not None:
                        av(*pending)
                    pending = (mi, m, e)
                av(*pending)

                if zrep:
                    # Psum rows 64:128 hold Z replicated: one [64,512]
                    # reciprocal per half straight into SBUF, then multiply
                    # straight out of the U psum into this pair's Ab half.
                    # LO chain first so the projection's n=0 matmuls can
                    # start before the HI chain finishes.
                    zbr = zbp.tile([64, N], F32, tag="zbr", name=f"zbr{h}")
                    o0 = half * 64
                    nc.vector.reciprocal(zbr[:, 0:512], uLO[64:128, :])
                    nc.vector.tensor_mul(Ab[tt][o0:o0 + 64, 0:512],
                                         uLO[0:64, :], zbr[:, 0:512])
                    nc.vector.reciprocal(zbr[:, 512:1024], uHI[64:128, :])
                    nc.vector.tensor_mul(Ab[tt][o0:o0 + 64, 512:1024],
                                         uHI[0:64, :], zbr[:, 512:1024])
                else:
                    # Drain: 1/Z straight from the psum Z rows first (it
                    # heads the pair's critical path; Z is O(1e3), the
                    # reference's +EPS is noise), then U rows to Ust.
                    zq[half] = zqp.tile([1, N], F32, tag=f"zq{half}",
                                        name=f"zq{tt}_{half}")
                    nc.vector.reciprocal(zq[half][0:1, 0:512], uLO[64:65, :])
                    nc.vector.reciprocal(zq[half][0:1, 512:1024],
                                         uHI[64:65, :])
                    Ust = UstA if half == 0 else UstB
                    if h == HPC - 1:
                        nc.scalar.activation(Ust[:, tt, 0:512], uLO[0:64, :],
                                             AF.Identity, bias=0.0, scale=1.0)
                    else:
                        nc.vector.tensor_copy(Ust[:, tt, 0:512], uLO[0:64, :])
                    nc.vector.tensor_copy(Ust[:, tt, 512:1024], uHI[0:64, :])

                    if half == 1:
                        # Broadcast 1/Z across partitions on the idle gpsimd
                        # engine (one 64-row tile per head) and normalize.
                        zbA = zbp.tile([64, N], F32, tag="zbA",
                                       name=f"zbA{tt}")
                        zbB = zbp.tile([64, N], F32, tag="zbB",
                                       name=f"zbB{tt}")
                        nc.gpsimd.partition_broadcast(zbA, zq[0][0:1, :])
                        nc.gpsimd.partition_broadcast(zbB, zq[1][0:1, :])
                        nc.vector.tensor_mul(Ab[tt][0:64, :], UstA[:, tt, :],
                                             zbA)
                        nc.vector.tensor_mul(Ab[tt][64:128, :],
                                             UstB[:, tt, :], zbB)

                # Remaining q/k blocks ride the exp-paced PE stream between
                # heads (their psum tiles rotate through the score pool);
                # q copies fill the activation engine's boundary idle.
                if h == 0:
                    qk_block(JSEQ[2], True)
                    qk_block(JSEQ[3], False)
                elif h == 1:
                    qk_block(JSEQ[4], True)
                    qk_block(JSEQ[5], False)

        # ---- output projection (partial, this head-group) ----
        # Psum tiles come from the score pool: its buffers free as soon as
        # the last head's exps retire, so the pair-0/1 contributions of the
        # first two columns run during the last pair's Z-normalize; the
        # pair-2 matmuls join once Ab[2] lands.
        with tc.tile_pool(name="op", bufs=3) as op, \
             tc.tile_pool(name="pj2", bufs=2, space="PSUM") as pj2:
            def pj_mm(ps, co, kt, n):
                nc.tensor.matmul(ps[:, n * 512:(n + 1) * 512],
                                 lhsT=projT_sb[:, kt, co * 128:(co + 1) * 128],
                                 rhs=Ab[kt][:, n * 512:(n + 1) * 512],
                                 start=(kt == 0), stop=(kt == 2))

            pre = {}
            for co in range(4):
                pool = stp if co < 2 else pj2
                tag = "st" if co < 2 else "pj"
                pre[co] = pool.tile([128, N], F32, tag=tag, name=f"pjps{co}")
                for kt in range(2):
                    for n in range(2):
                        pj_mm(pre[co], co, kt, n)
            for co in range(CO):
                if co in pre:
                    ps = pre[co]
                    for n in range(2):
                        pj_mm(ps, co, 2, n)
                else:
                    ps = stp.tile([128, N], F32, tag="st", name=f"pjps{co}")
                    for kt in range(3):
                        for n in range(2):
                            pj_mm(ps, co, kt, n)
                ot = op.tile([128, N], F32, tag="o")
                if co < CO - 1:
                    if co % 2 == 0:
                        nc.vector.tensor_scalar(out=ot, in0=ps,
                                                scalar1=pbias_sb[:, co:co + 1],
                                                scalar2=None, op0=ALU.add)
                    else:
                        nc.scalar.activation(ot, ps, AF.Identity,
                                             bias=pbias_sb[:, co:co + 1],
                                             scale=1.0)
                    nc.sync.dma_start(out=t["outT"].ap()[:, co, :], in_=ot)
                else:
                    # Last column: halves on both engines, DMA per half.
                    nc.scalar.activation(ot[:, 0:512], ps[:, 0:512],
                                         AF.Identity,
                                         bias=pbias_sb[:, co:co + 1],
                                         scale=1.0)
                    nc.sync.dma_start(out=t["outT"].ap()[:, co, 0:512],
                                      in_=ot[:, 0:512])
                    nc.vector.tensor_scalar(out=ot[:, 512:1024],
                                            in0=ps[:, 512:1024],
                                            scalar1=pbias_sb[:, co:co + 1],
                                            scalar2=None, op0=ALU.add)
                    nc.sync.dma_start(out=t["outT"].ap()[:, co, 512:1024],
                                      in_=ot[:, 512:1024])


_NC_CACHE = {}


def build_nc(reps: int = 1, with_vbias: bool = True, loop: int = 0,
             dummies: bool = False, zrep: bool = False):
    key = (reps, with_vbias, loop, dummies, zrep)
    if key in _NC_CACHE:
        return _NC_CACHE[key]
    nc = bacc.Bacc("TRN2", target_bir_lowering=False, debug=False,
                   num_devices=N_CORES)
    t = {
        "xT": nc.dram_tensor("xT", [128, KT, N], BF16, kind="ExternalInput"),
        "wqkT": nc.dram_tensor("wqkT", [128, KT, KT, 128], BF16,
                               kind="ExternalInput"),
        "qkb": nc.dram_tensor("qkb", [128, KT], F32, kind="ExternalInput"),
        "wvT": nc.dram_tensor("wvT", [128, KT, HPC * HD], BF16,
                              kind="ExternalInput"),
        "vbias": nc.dram_tensor("vbias", [1, HPC * HD], BF16,
                                kind="ExternalInput"),
        "ones1": nc.dram_tensor("ones1", [1, 128], BF16, kind="ExternalInput"),
        "mq": nc.dram_tensor("mq", [2, N], BF16, kind="ExternalInput"),
        "mk": nc.dram_tensor("mk", [2, N], BF16, kind="ExternalInput"),
        "projT": nc.dram_tensor("projT", [128, 3, C], BF16,
                                kind="ExternalInput"),
        "pbias": nc.dram_tensor("pbias", [128, CO], F32, kind="ExternalInput"),
        "outT": nc.dram_tensor("outT", [128, CO, N], F32,
                               kind="ExternalOutput"),
    }
    with tile.TileContext(nc) as tc:
        if loop:
            with tc.For_i(0, loop, 1):
                _body(nc, tc, t, with_vbias=with_vbias, dummies=dummies,
                      zrep=zrep)
        else:
            for _ in range(reps):
                _body(nc, tc, t, with_vbias=with_vbias, dummies=dummies,
                      zrep=zrep)
    nc.compile()
    _NC_CACHE[key] = nc
    return nc


def _is_onehot(decision: np.ndarray) -> bool:
    vals_ok = np.all((decision == 0.0) | (decision == 1.0))
    return bool(vals_ok and np.all(decision.sum(-1) == 1.0))


def _perms(decision):
    """Per-batch token permutation [g0 | g1-tail + g2 | g1-pure-256]."""
    perms = []
    for b in range(B):
        d0 = decision[b][:, 0]
        g1 = G0 + np.where(d0 == 1.0)[0]
        g2 = G0 + np.where(d0 != 1.0)[0]
        if len(g1) < 256 or len(g2) < 1:
            return None
        srt = np.concatenate([np.arange(G0), g1, g2])
        perms.append(np.concatenate([srt[0:256], srt[512:1024], srt[256:512]]))
    return perms


def make_in_maps(x, decision, qkv_w, qkv_b, proj_w, proj_b, S, perms):
    in_maps = []
    xT_cache = {}
    ones1 = np.ones((1, 128), ml_dtypes.bfloat16)
    for core in range(N_CORES):
        b, hg = core // 2, core % 2
        pi = perms[b]
        if b not in xT_cache:
            xT = np.ascontiguousarray(x[b][pi].T)  # [C, N]
            xT_cache[b] = xT.reshape(KT, 128, N).transpose(1, 0, 2).astype(
                ml_dtypes.bfloat16)
        qs = slice(hg * 384, hg * 384 + 384)
        ks = slice(C + hg * 384, C + hg * 384 + 384)
        vs = slice(2 * C + hg * 384, 2 * C + hg * 384 + 384)
        Wqk = np.concatenate([qkv_w[qs] * SCALE, qkv_w[ks]], axis=0)  # [768,C]
        # [128, jblock, kt, 128]: lhsT layout, contraction partition-major
        wqkT = Wqk.T.reshape(KT, 128, KT, 128).transpose(1, 2, 0, 3).astype(
            ml_dtypes.bfloat16)
        bqk = np.concatenate([qkv_b[qs] * SCALE, qkv_b[ks]])
        qkb = np.ascontiguousarray(bqk.reshape(KT, 128).T, dtype=np.float32)
        wvT = qkv_w[vs].T.reshape(KT, 128, 384).transpose(1, 0, 2).astype(
            ml_dtypes.bfloat16)
        vbias = qkv_b[vs].reshape(1, 384).astype(ml_dtypes.bfloat16)
        p0 = np.zeros(N, np.float32)
        p0[:G0] = 1.0
        p1 = np.zeros(N, np.float32)
        p1[G0:] = decision[b][:, 0]
        p0, p1 = p0[pi], p1[pi]
        mq = np.stack([p1, p0]).astype(ml_dtypes.bfloat16)
        mk = np.stack([-BIG * p0, -BIG * p1]).astype(ml_dtypes.bfloat16)
        projT = (proj_w[:, hg * 384:hg * 384 + 384].T
                 .reshape(3, 128, C).transpose(1, 0, 2)
                 .astype(ml_dtypes.bfloat16))
        if hg == 0:
            pbias = np.ascontiguousarray(
                proj_b.reshape(CO, 128).T, dtype=np.float32)
        else:
            pbias = np.zeros((128, CO), np.float32)
        in_maps.append({
            "xT": xT_cache[b], "wqkT": wqkT, "qkb": qkb, "wvT": wvT,
            "vbias": vbias, "ones1": ones1, "mq": mq, "mk": mk,
            "projT": projT, "pbias": pbias,
        })
    return in_maps


def _numpy_fallback(x, decision, qkv_w, qkv_b, proj_w, proj_b, S):
    """Direct port of the reference for non-one-hot policies."""
    out = np.empty((B, N, C), np.float32)
    for b in range(B):
        p0 = np.zeros(N, np.float32)
        p0[:N - S] = 1.0
        p1 = np.zeros(N, np.float32)
        p1[N - S:] = decision[b][:, 0]
        p2 = np.zeros(N, np.float32)
        p2[N - S:] = decision[b][:, 1]
        qkv = x[b] @ qkv_w.T + qkv_b
        qkv = qkv.reshape(N, 3, H, HD).transpose(1, 2, 0, 3)
        q, k, v = qkv[0], qkv[1], qkv[2]          # [H, N, HD]
        s = p0 + p1 + p2
        ap = (np.outer(s, s) - np.outer(p0, p1) - np.outer(p1, p0))
        ap = ap + (1.0 - ap) * np.eye(N, dtype=np.float32)
        attn = np.einsum('hnd,hmd->hnm', q, k).astype(np.float32) * SCALE
        m = attn.max(-1, keepdims=True)
        e = np.exp(attn - m) * ap[None]
        p = (e + EPS / N) / (e.sum(-1, keepdims=True) + EPS)
        o = np.einsum('hnm,hmd->hnd', p, v)
        out[b] = o.transpose(1, 0, 2).reshape(N, C) @ proj_w.T + proj_b
    return out


def kernel(x, decision, qkv_w, qkv_b, proj_w, proj_b, search_feat_len):
    x = np.asarray(x, np.float32)
    decision = np.asarray(decision, np.float32)
    qkv_w = np.asarray(qkv_w, np.float32)
    qkv_b = np.asarray(qkv_b, np.float32)
    proj_w = np.asarray(proj_w, np.float32)
    proj_b = np.asarray(proj_b, np.float32)
    S = int(np.asarray(search_feat_len))

    perms = _perms(decision) if _is_onehot(decision) and S == 768 else None
    if perms is None:
        return _numpy_fallback(x, decision, qkv_w, qkv_b, proj_w, proj_b, S)

    nc = build_nc(with_vbias=bool(np.any(qkv_b[2 * C:] != 0.0)), zrep=True)
    in_maps = make_in_maps(x, decision, qkv_w, qkv_b, proj_w, proj_b, S,
                           perms)
    res = run_bass_kernel_spmd(nc, in_maps, core_ids=list(range(N_CORES)))

    out = np.empty((B, N, C), np.float32)
    for b in range(B):
        partial = res.results[2 * b]["outT"] + res.results[2 * b + 1]["outT"]
        out[b][perms[b]] = partial.transpose(1, 0, 2).reshape(C, N).T
    return out
